# revision 29
# baseline (speedup 1.0000x reference)
"""Trainium2 Bass kernel for the BRBboxHead problem.

Computes, for fused_feats [32, 256, 4096]:
    h  = relu(BN0(W0 @ x))          (1x1 conv + BN folded on host)
    h  = relu(BN1(W1 @ h))
    cls = Wc @ h + bc               -> sem_scores [32, 4096, 18]
    reg = Wr @ h + br
    refined_angle    = coarse_angle + reg[0]
    refined_distance = coarse_distance + reg[1:7]

Sharding: data-parallel over batch, 4 batches per core across 8 cores.
Weights are folded host-side (BN scale/shift into W/b) and replicated.

Device pipeline per 512-position tile (channel-major trunk):
    DMA x [128, 2x512] -> PE matmuls (K=256 via 2-step PSUM accum)
    -> ACT relu+bias -> PE matmul (K=128) -> ACT relu+bias
    -> PE heads matmul [128,32]^T @ h1 -> [32, 512] PSUM
    -> ACT identity+bias(bc|br|0) -> DVE 32x32 stream transpose
    -> DVE add of coarse residuals on reg columns -> contiguous DMA out.

All DRAM-side layouts are chosen so every DMA is a contiguous (or
2KB-row) dump; the host does the cheap permutes in numpy.
"""
import os
import sys

sys.path.insert(0, '/opt/trn_rl_repo')

import numpy as np

import concourse.bass as bass
import concourse.mybir as mybir
import concourse.tile as tile
from concourse.bass_utils import run_bass_kernel_spmd
from concourse.tile_sem_assignment import N_PROCS
from concourse.vector_clock import ScopedClock, VectorClock

# Problem shapes (hardcoded per contest contract).
B, CIN, N = 32, 256, 4096
C1 = 128
NCLS, NREG = 18, 7
CH = 32               # head channels padded 25 -> 32 for the stream transpose
N_CORES = 8
BPC = B // N_CORES    # batches per core
NT = 512              # positions per tile
NTILES = N // NT
BN_EPS = 1e-5

F32 = mybir.dt.float32
# Matmul dtype for trunk inputs/weights:
#   f32   - exact, 4 cyc/row PE stream
#   f32r  - TF32-like, ~2 cyc/row
#   bf16  - 1 cyc/row and halves the x DMA bytes (host casts)
_MM_DT = {
    "f32": mybir.dt.float32,
    "f32r": mybir.dt.float32r,
    "bf16": mybir.dt.bfloat16,
}[os.environ.get("BRB_MM_DT", "bf16")]
# fp32r/fp32 can't col-group the heads matmul (invalid ISA with
# tile_position); they fall back to a per-tile epilogue
_QUAD = _MM_DT == mybir.dt.bfloat16


def _split_multi_waits(nc):
    """The walrus build here rejects instructions carrying more than one
    sync wait. Hoist all but the last wait of each instruction onto NOPs
    inserted just before it on the same engine — engines execute in
    order, so waiting on each sem in sequence is equivalent."""
    for f in nc.m.functions:
        for bb in f.blocks:
            out = []
            changed = False
            for inst in list(bb.instructions):
                si = inst.sync_info
                if si is not None and len(si.on_wait) > 1:
                    waits = list(si.on_wait)
                    for w in waits[:-1]:
                        nop = mybir.InstNoOp(name=nc.get_next_instruction_name())
                        nop.engine = inst.engine
                        nop.sync_info = mybir.SyncInfo(on_wait=[w], on_update=[])
                        out.append(nop)
                    inst.sync_info = mybir.SyncInfo(
                        on_wait=[waits[-1]], on_update=list(si.on_update)
                    )
                    changed = True
                out.append(inst)
            if changed:
                bb.instructions = out


def _build_program():
    nc = bass.Bass("TRN2", target_bir_lowering=False, debug=False)

    # x laid out [BPC, 2, 128, N]: K-chunk-major, dtype matches the matmul
    x = nc.dram_tensor("x", [BPC, 2, 128, N], _MM_DT, kind="ExternalInput").ap()
    # coarse residuals pre-arranged to the post-transpose SBUF layout
    if _QUAD:
        cc = nc.dram_tensor(
            "cc", [BPC, 128, (NTILES // 4) * (NT // 32) * 25], F32,
            kind="ExternalInput"
        ).ap()
    else:
        cc = nc.dram_tensor(
            "cc", [BPC, 32, NTILES * (NT // 32) * 25], F32,
            kind="ExternalInput"
        ).ap()
    w0a = nc.dram_tensor("w0a", [128, C1], _MM_DT, kind="ExternalInput").ap()
    w0b = nc.dram_tensor("w0b", [128, C1], _MM_DT, kind="ExternalInput").ap()
    w1 = nc.dram_tensor("w1", [C1, C1], _MM_DT, kind="ExternalInput").ap()
    wh = nc.dram_tensor("wh", [C1, CH], _MM_DT, kind="ExternalInput").ap()
    b0 = nc.dram_tensor("b0", [C1, 1], F32, kind="ExternalInput").ap()
    b1 = nc.dram_tensor("b1", [C1, 1], F32, kind="ExternalInput").ap()

    # raw transposed head tiles; host unpacks to sem/angle/distance
    if _QUAD:
        out_o = nc.dram_tensor(
            "out", [BPC, 128, N // 4], F32, kind="ExternalOutput"
        ).ap()
    else:
        out_o = nc.dram_tensor(
            "out", [BPC, CH, N], F32, kind="ExternalOutput"
        ).ap()

    relu = mybir.ActivationFunctionType.Relu
    ident = mybir.ActivationFunctionType.Identity

    with tile.TileContext(nc) as tc:
        with (
            tc.tile_pool(name="consts", bufs=1) as consts,
            tc.tile_pool(name="xin", bufs=4) as xpool,
            tc.tile_pool(name="hmid", bufs=4) as hpool,
            tc.tile_pool(name="tout", bufs=3) as tpool,
            tc.tile_pool(name="oout", bufs=3) as opool,
            tc.tile_pool(name="ccin", bufs=3) as ccpool,
            tc.tile_pool(name="ps_l0", bufs=2, space="PSUM") as psl0,
            tc.tile_pool(name="ps_l1", bufs=1, space="PSUM") as psl1,
            tc.tile_pool(name="ps_head", bufs=2, space="PSUM") as pshead,
        ):
            # w0a/w0b ride the sync ring ahead of x: the sync sequencer has
            # ~3us less preamble than scalar, so the HAM warmup can start early
            w0a_sb = consts.tile([128, C1], _MM_DT, tag="w0a")
            nc.sync.dma_start(w0a_sb[:], w0a[:])
            w0b_sb = consts.tile([128, C1], _MM_DT, tag="w0b")
            nc.sync.dma_start(w0b_sb[:], w0b[:])
            w1_sb = consts.tile([C1, C1], _MM_DT, tag="w1")
            nc.scalar.dma_start(w1_sb[:], w1[:])
            wh_sb = consts.tile([C1, CH], _MM_DT, tag="wh")
            nc.scalar.dma_start(wh_sb[:], wh[:])
            b0_sb = consts.tile([C1, 1], F32, tag="b0")
            nc.scalar.dma_start(b0_sb[:], b0[:])
            b1_sb = consts.tile([C1, 1], F32, tag="b1")
            nc.scalar.dma_start(b1_sb[:], b1[:])

            # HAM warmup: ~3.5us of dummy matmuls on resident weights while
            # the PE would otherwise idle waiting for x. Gets the clock gate
            # to K=8/8 before the first real matmul lands.
            if _QUAD:
                warm = pshead.tile([128, NT], F32, tag="ph")
                for _ in range(32):
                    nc.tensor.matmul(warm[:, 0:128], w0a_sb[:], w0b_sb[:])

            CCW = (NT // 32) * 25   # 400 cc columns per tile
            for b in range(BPC):
                xbig = xpool.tile([128, 2 * N], _MM_DT, tag="xt")
                # j-interleaved ascending chunk sizes: the first tiles'
                # K-chunks land quickly, later chunks amortize trigger cost
                for c0, c1 in ((0, 512), (512, 2048), (2048, N)):
                    for j in range(2):
                        nc.sync.dma_start(
                            xbig[:, j * N + c0:j * N + c1],
                            x[b, j, :, c0:c1],
                        )
                GCW = (NT // 32) * 25  # 400 cc columns per quad-group
                if _QUAD:
                    ccb = ccpool.tile([128, (NTILES // 4) * GCW], F32, tag="cc")
                    outb = opool.tile([128, N // 4], F32, tag="outb")
                else:
                    ccb = ccpool.tile([32, NTILES * GCW], F32, tag="cc")
                    outb = opool.tile([CH, N], F32, tag="outb")
                nc.gpsimd.dma_start(ccb[:], cc[b])

                for g in range(NTILES // 4):
                    # heads output for 4 tiles lands in one PSUM bank,
                    # one col-group (32 partitions) per tile
                    if _QUAD:
                        ph4 = pshead.tile([128, NT], F32, tag="ph")
                    else:
                        ph4 = None
                    for half in range(2):
                        tp = 2 * g + half
                        # L0 over a tile pair -> one [128, 1024] PSUM tile
                        p0 = psl0.tile([128, 2 * NT], F32, tag="p0")
                        h0 = hpool.tile([128, 2 * NT], _MM_DT, tag="h0")
                        for w_sb, joff, st in (
                            (w0a_sb, 0, True), (w0b_sb, N, False)
                        ):
                            for u in range(2):
                                n0 = (2 * tp + u) * NT
                                nc.tensor.matmul(
                                    p0[:, u * NT:(u + 1) * NT], w_sb[:],
                                    xbig[:, joff + n0:joff + n0 + NT],
                                    start=st, stop=not st,
                                )
                        if half == 0:
                            nc.scalar.activation(
                                h0[:], p0[:], relu, bias=b0_sb[:, 0:1]
                            )
                        else:
                            nc.vector.tensor_scalar(
                                h0[:], p0[:], b0_sb[:, 0:1], 0.0,
                                mybir.AluOpType.add, mybir.AluOpType.max,
                            )

                        p1 = psl1.tile([128, 2 * NT], F32, tag="p1")
                        nc.tensor.matmul(
                            p1[:, 0:NT], w1_sb[:], h0[:, 0:NT]
                        )
                        nc.tensor.matmul(
                            p1[:, NT:2 * NT], w1_sb[:], h0[:, NT:2 * NT]
                        )
                        h1 = hpool.tile([128, 2 * NT], _MM_DT, tag="h1")
                        nc.scalar.activation(
                            h1[:], p1[:], relu, bias=b1_sb[:, 0:1]
                        )

                        for u in range(2):
                            k = 2 * half + u  # col-group = tile index in quad
                            if _QUAD:
                                nc.tensor.matmul(
                                    ph4[:][32 * k:32 * (k + 1), :], wh_sb[:],
                                    h1[:, u * NT:(u + 1) * NT],
                                    tile_position=(0, 32 * k),
                                )
                            else:
                                t = 4 * g + k
                                n0 = t * NT
                                ph = pshead.tile([CH, NT], F32, tag="ph1")
                                nc.tensor.matmul(
                                    ph[:], wh_sb[:],
                                    h1[:, u * NT:(u + 1) * NT],
                                )
                                nc.vector.transpose(outb[:, n0:n0 + NT], ph[:])
                                tyv = outb[:, n0:n0 + NT].rearrange(
                                    "p (m c) -> p m c", c=32
                                )[:, :, 0:25]
                                ccv = ccb[:, t * GCW:(t + 1) * GCW].rearrange(
                                    "p (m c) -> p m c", c=25
                                )
                                nc.vector.tensor_add(tyv, tyv, ccv)

                    if not _QUAD:
                        continue
                    # one 32x32 block transpose covers all 4 tiles
                    nc.vector.transpose(outb[:, g * NT:(g + 1) * NT], ph4[:])

                    tyv = outb[:, g * NT:(g + 1) * NT].rearrange(
                        "p (m c) -> p m c", c=32
                    )[:, :, 0:25]
                    ccv = ccb[:, g * GCW:(g + 1) * GCW].rearrange(
                        "p (m c) -> p m c", c=25
                    )
                    nc.vector.tensor_add(tyv, tyv, ccv)

                nc.gpsimd.dma_start(out_o[b], outb[:])
    if os.environ.get("BRB_SKIP_SPLIT") != "1":
        _split_multi_waits(nc)
    return nc


_NC_CACHE = {}


def _get_program():
    key = str(_MM_DT)
    if key not in _NC_CACHE:
        _NC_CACHE[key] = _build_program()
    return _NC_CACHE[key]


def _host_fold(inputs):
    """Fold BN into conv weights/biases; build per-core input maps."""
    f = lambda k: np.asarray(inputs[k], dtype=np.float32)
    W0, b0, g0, be0, m0, v0 = (f(k) for k in ("W0", "b0", "g0", "be0", "m0", "v0"))
    W1, b1, g1, be1, m1, v1 = (f(k) for k in ("W1", "b1", "g1", "be1", "m1", "v1"))
    Wc, bc, Wr, br = (f(k) for k in ("Wc", "bc", "Wr", "br"))

    s0 = g0 / np.sqrt(v0 + BN_EPS)
    W0f = W0 * s0[:, None]
    b0f = (b0 - m0) * s0 + be0
    s1 = g1 / np.sqrt(v1 + BN_EPS)
    W1f = W1 * s1[:, None]
    b1f = (b1 - m1) * s1 + be1

    w0T = np.ascontiguousarray(W0f.T)            # [256, 128]
    wh = np.zeros((C1, CH), np.float32)          # [128, 32]
    wh[:, 0:NCLS] = Wc.T
    wh[:, NCLS:25] = Wr.T


    mm_np = mybir.dt.np(_MM_DT)
    fused = np.asarray(inputs["fused_feats"], dtype=np.float32)
    # [B, 256, N] -> [B, 2, 128, N] K-chunk split (a reshape, no copy)
    fused = fused.reshape(B, 2, 128, N).astype(mm_np)

    angle = np.asarray(inputs["coarse_angle"], dtype=np.float32)
    dist = np.asarray(inputs["coarse_distance"], dtype=np.float32)
    cc = np.empty((B, N, 25), np.float32)
    cc[:, :, 0:NCLS] = bc[None, None, :]
    cc[:, :, NCLS] = angle + br[0]
    cc[:, :, NCLS + 1:25] = dist + br[None, None, 1:]
    if _QUAD:
        # cc_dev[b, 32k+p, g*400 + m*25 + c] = cc[b, (4g+k)*512 + 32m + p, c]
        cc_dev = np.ascontiguousarray(
            cc.reshape(B, NTILES // 4, 4, NT // 32, 32, 25).transpose(
                0, 2, 4, 1, 3, 5
            )
        ).reshape(B, 128, (NTILES // 4) * (NT // 32) * 25)
    else:
        # cc_dev[b, p, t*400 + m*25 + c] = cc[b, t*512 + 32m + p, c]
        cc_dev = np.ascontiguousarray(
            cc.reshape(B, NTILES, NT // 32, 32, 25).transpose(0, 3, 1, 2, 4)
        ).reshape(B, 32, NTILES * (NT // 32) * 25)

    shared = {
        "w0a": np.ascontiguousarray(w0T[0:128]).astype(mm_np),
        "w0b": np.ascontiguousarray(w0T[128:256]).astype(mm_np),
        "w1": np.ascontiguousarray(W1f.T).astype(mm_np),
        "wh": wh.astype(mm_np),
        "b0": b0f.reshape(C1, 1),
        "b1": b1f.reshape(C1, 1),
    }
    in_maps = []
    for c in range(N_CORES):
        lo, hi = c * BPC, (c + 1) * BPC
        m = dict(shared)
        m["x"] = np.ascontiguousarray(fused[lo:hi])
        m["cc"] = np.ascontiguousarray(cc_dev[lo:hi])
        in_maps.append(m)
    return in_maps


def _run(inputs, trace=False):
    nc = _get_program()
    in_maps = _host_fold(inputs)
    res = run_bass_kernel_spmd(
        nc, in_maps, core_ids=list(range(N_CORES)), trace=trace
    )
    raw = np.concatenate([res.results[c]["out"] for c in range(N_CORES)], axis=0)
    if _QUAD:
        # raw[b, 32k+p, g*512+32m+c] = channel c at pos (4g+k)*512 + 32m + p
        full = np.ascontiguousarray(
            raw.reshape(B, 4, 32, NTILES // 4, NT // 32, 32).transpose(
                0, 3, 1, 4, 2, 5
            )
        ).reshape(B, N, 32)
    else:
        # raw[b, p, t*512 + 32m + c] = channel c at pos t*512 + 32m + p
        full = np.ascontiguousarray(
            raw.reshape(B, 32, NTILES, NT // 32, 32).transpose(0, 2, 3, 1, 4)
        ).reshape(B, N, 32)
    sem_scores = np.ascontiguousarray(full[:, :, 0:NCLS])
    refined_angle = np.ascontiguousarray(full[:, :, NCLS])
    refined_distance = np.ascontiguousarray(full[:, :, NCLS + 1:25])
    return (sem_scores, refined_angle, refined_distance), res


def kernel(**inputs):
    outs, _ = _run(inputs, trace=False)
    return outs


# revision 30
# speedup vs baseline: 1.0842x; 1.0842x over previous
"""Trainium2 Bass kernel for the BRBboxHead problem.

Computes, for fused_feats [32, 256, 4096]:
    h  = relu(BN0(W0 @ x))          (1x1 conv + BN folded on host)
    h  = relu(BN1(W1 @ h))
    cls = Wc @ h + bc               -> sem_scores [32, 4096, 18]
    reg = Wr @ h + br
    refined_angle    = coarse_angle + reg[0]
    refined_distance = coarse_distance + reg[1:7]

Sharding: data-parallel over batch, 4 batches per core across 8 cores.
Weights are folded host-side (BN scale/shift into W/b) and replicated.

Device pipeline per 512-position tile (channel-major trunk):
    DMA x [128, 2x512] -> PE matmuls (K=256 via 2-step PSUM accum)
    -> ACT relu+bias -> PE matmul (K=128) -> ACT relu+bias
    -> PE heads matmul [128,32]^T @ h1 -> [32, 512] PSUM
    -> ACT identity+bias(bc|br|0) -> DVE 32x32 stream transpose
    -> DVE add of coarse residuals on reg columns -> contiguous DMA out.

All DRAM-side layouts are chosen so every DMA is a contiguous (or
2KB-row) dump; the host does the cheap permutes in numpy.
"""
import os
import sys

sys.path.insert(0, '/opt/trn_rl_repo')

import numpy as np

import concourse.bass as bass
import concourse.mybir as mybir
import concourse.tile as tile
from concourse.bass_utils import run_bass_kernel_spmd
from concourse.tile_sem_assignment import N_PROCS
from concourse.vector_clock import ScopedClock, VectorClock

# Problem shapes (hardcoded per contest contract).
B, CIN, N = 32, 256, 4096
C1 = 128
NCLS, NREG = 18, 7
CH = 32               # head channels padded 25 -> 32 for the stream transpose
N_CORES = 8
BPC = B // N_CORES    # batches per core
NT = 512              # positions per tile
NTILES = N // NT
BN_EPS = 1e-5

F32 = mybir.dt.float32
# Matmul dtype for trunk inputs/weights:
#   f32   - exact, 4 cyc/row PE stream
#   f32r  - TF32-like, ~2 cyc/row
#   bf16  - 1 cyc/row and halves the x DMA bytes (host casts)
_MM_DT = {
    "f32": mybir.dt.float32,
    "f32r": mybir.dt.float32r,
    "bf16": mybir.dt.bfloat16,
}[os.environ.get("BRB_MM_DT", "bf16")]
# fp32r/fp32 can't col-group the heads matmul (invalid ISA with
# tile_position); they fall back to a per-tile epilogue
_QUAD = _MM_DT == mybir.dt.bfloat16


def _split_multi_waits(nc):
    """The walrus build here rejects instructions carrying more than one
    sync wait. Hoist all but the last wait of each instruction onto NOPs
    inserted just before it on the same engine — engines execute in
    order, so waiting on each sem in sequence is equivalent."""
    for f in nc.m.functions:
        for bb in f.blocks:
            out = []
            changed = False
            for inst in list(bb.instructions):
                si = inst.sync_info
                if si is not None and len(si.on_wait) > 1:
                    waits = list(si.on_wait)
                    for w in waits[:-1]:
                        nop = mybir.InstNoOp(name=nc.get_next_instruction_name())
                        nop.engine = inst.engine
                        nop.sync_info = mybir.SyncInfo(on_wait=[w], on_update=[])
                        out.append(nop)
                    inst.sync_info = mybir.SyncInfo(
                        on_wait=[waits[-1]], on_update=list(si.on_update)
                    )
                    changed = True
                out.append(inst)
            if changed:
                bb.instructions = out


def _build_program():
    nc = bass.Bass("TRN2", target_bir_lowering=False, debug=False)

    # x laid out [BPC, 2, 128, N]: K-chunk-major, dtype matches the matmul
    x = nc.dram_tensor("x", [BPC, 2, 128, N], _MM_DT, kind="ExternalInput").ap()
    # coarse residuals pre-arranged to the post-transpose SBUF layout
    if _QUAD:
        cc = nc.dram_tensor(
            "cc", [BPC, 128, (NTILES // 4) * (NT // 32) * 25], F32,
            kind="ExternalInput"
        ).ap()
    else:
        cc = nc.dram_tensor(
            "cc", [BPC, 32, NTILES * (NT // 32) * 25], F32,
            kind="ExternalInput"
        ).ap()
    w0a = nc.dram_tensor("w0a", [128, C1], _MM_DT, kind="ExternalInput").ap()
    w0b = nc.dram_tensor("w0b", [128, C1], _MM_DT, kind="ExternalInput").ap()
    w1 = nc.dram_tensor("w1", [C1, C1], _MM_DT, kind="ExternalInput").ap()
    wh = nc.dram_tensor("wh", [C1, CH], _MM_DT, kind="ExternalInput").ap()
    b0 = nc.dram_tensor("b0", [C1, 1], F32, kind="ExternalInput").ap()
    b1 = nc.dram_tensor("b1", [C1, 1], F32, kind="ExternalInput").ap()

    # raw transposed head tiles; host unpacks to sem/angle/distance
    if _QUAD:
        out_o = nc.dram_tensor(
            "out", [BPC, 128, N // 4], F32, kind="ExternalOutput"
        ).ap()
    else:
        out_o = nc.dram_tensor(
            "out", [BPC, CH, N], F32, kind="ExternalOutput"
        ).ap()

    relu = mybir.ActivationFunctionType.Relu
    ident = mybir.ActivationFunctionType.Identity

    with tile.TileContext(nc) as tc:
        with (
            tc.tile_pool(name="consts", bufs=1) as consts,
            tc.tile_pool(name="xin", bufs=3) as xpool,
            tc.tile_pool(name="hmid", bufs=3) as hpool,
            tc.tile_pool(name="tout", bufs=3) as tpool,
            tc.tile_pool(name="oout", bufs=2) as opool,
            tc.tile_pool(name="ccin", bufs=2) as ccpool,
            tc.tile_pool(name="ps_l0", bufs=2, space="PSUM") as psl0,
            tc.tile_pool(name="ps_l1", bufs=1, space="PSUM") as psl1,
            tc.tile_pool(name="ps_head", bufs=2, space="PSUM") as pshead,
        ):
            # w0a/w0b ride the sync ring ahead of x: the sync sequencer has
            # ~3us less preamble than scalar, so the HAM warmup can start early
            w0a_sb = consts.tile([128, C1], _MM_DT, tag="w0a")
            nc.sync.dma_start(w0a_sb[:], w0a[:])
            w0b_sb = consts.tile([128, C1], _MM_DT, tag="w0b")
            nc.sync.dma_start(w0b_sb[:], w0b[:])
            w1_sb = consts.tile([C1, C1], _MM_DT, tag="w1")
            nc.scalar.dma_start(w1_sb[:], w1[:])
            wh_sb = consts.tile([C1, CH], _MM_DT, tag="wh")
            nc.scalar.dma_start(wh_sb[:], wh[:])
            b0_sb = consts.tile([C1, 1], F32, tag="b0")
            nc.scalar.dma_start(b0_sb[:], b0[:])
            b1_sb = consts.tile([C1, 1], F32, tag="b1")
            nc.scalar.dma_start(b1_sb[:], b1[:])

            # HAM warmup: ~3.5us of dummy matmuls on resident weights while
            # the PE would otherwise idle waiting for x. Gets the clock gate
            # to K=8/8 before the first real matmul lands.
            if _QUAD:
                warm = pshead.tile([128, NT], F32, tag="ph")
                for _ in range(32):
                    nc.tensor.matmul(warm[:, 0:128], w0a_sb[:], w0b_sb[:])

            CCW = (NT // 32) * 25   # 400 cc columns per tile
            for b in range(BPC):
                xbig = xpool.tile([128, 2 * N], _MM_DT, tag="xt")
                # j-interleaved ascending chunk sizes: the first tiles'
                # K-chunks land quickly, later chunks amortize trigger cost
                for c0, c1 in ((0, 512), (512, 2048), (2048, N)):
                    for j in range(2):
                        nc.sync.dma_start(
                            xbig[:, j * N + c0:j * N + c1],
                            x[b, j, :, c0:c1],
                        )
                GCW = (NT // 32) * 25  # 400 cc columns per quad-group
                if _QUAD:
                    ccb = ccpool.tile([128, (NTILES // 4) * GCW], F32, tag="cc")
                    outb = opool.tile([128, N // 4], F32, tag="outb")
                else:
                    ccb = ccpool.tile([32, NTILES * GCW], F32, tag="cc")
                    outb = opool.tile([CH, N], F32, tag="outb")
                nc.gpsimd.dma_start(ccb[:], cc[b])

                for g in range(NTILES // 4):
                    # heads output for 4 tiles lands in one PSUM bank,
                    # one col-group (32 partitions) per tile
                    if _QUAD:
                        ph4 = pshead.tile([128, NT], F32, tag="ph")
                    else:
                        ph4 = None
                    for half in range(2):
                        tp = 2 * g + half
                        # L0 over a tile pair -> one [128, 1024] PSUM tile
                        p0 = psl0.tile([128, 2 * NT], F32, tag="p0")
                        h0 = hpool.tile([128, 2 * NT], _MM_DT, tag="h0")
                        for w_sb, joff, st in (
                            (w0a_sb, 0, True), (w0b_sb, N, False)
                        ):
                            for u in range(2):
                                n0 = (2 * tp + u) * NT
                                nc.tensor.matmul(
                                    p0[:, u * NT:(u + 1) * NT], w_sb[:],
                                    xbig[:, joff + n0:joff + n0 + NT],
                                    start=st, stop=not st,
                                )
                        if half == 0:
                            nc.scalar.activation(
                                h0[:], p0[:], relu, bias=b0_sb[:, 0:1]
                            )
                        else:
                            nc.vector.tensor_scalar(
                                h0[:], p0[:], b0_sb[:, 0:1], 0.0,
                                mybir.AluOpType.add, mybir.AluOpType.max,
                            )

                        p1 = psl1.tile([128, 2 * NT], F32, tag="p1")
                        nc.tensor.matmul(
                            p1[:, 0:NT], w1_sb[:], h0[:, 0:NT]
                        )
                        nc.tensor.matmul(
                            p1[:, NT:2 * NT], w1_sb[:], h0[:, NT:2 * NT]
                        )
                        h1 = hpool.tile([128, 2 * NT], _MM_DT, tag="h1")
                        nc.scalar.activation(
                            h1[:], p1[:], relu, bias=b1_sb[:, 0:1]
                        )

                        for u in range(2):
                            k = 2 * half + u  # col-group = tile index in quad
                            if _QUAD:
                                nc.tensor.matmul(
                                    ph4[:][32 * k:32 * (k + 1), :], wh_sb[:],
                                    h1[:, u * NT:(u + 1) * NT],
                                    tile_position=(0, 32 * k),
                                )
                            else:
                                t = 4 * g + k
                                n0 = t * NT
                                ph = pshead.tile([CH, NT], F32, tag="ph1")
                                nc.tensor.matmul(
                                    ph[:], wh_sb[:],
                                    h1[:, u * NT:(u + 1) * NT],
                                )
                                nc.vector.transpose(outb[:, n0:n0 + NT], ph[:])
                                tyv = outb[:, n0:n0 + NT].rearrange(
                                    "p (m c) -> p m c", c=32
                                )[:, :, 0:25]
                                ccv = ccb[:, t * GCW:(t + 1) * GCW].rearrange(
                                    "p (m c) -> p m c", c=25
                                )
                                nc.vector.tensor_add(tyv, tyv, ccv)

                    if not _QUAD:
                        continue
                    # one 32x32 block transpose covers all 4 tiles
                    nc.vector.transpose(outb[:, g * NT:(g + 1) * NT], ph4[:])

                    tyv = outb[:, g * NT:(g + 1) * NT].rearrange(
                        "p (m c) -> p m c", c=32
                    )[:, :, 0:25]
                    ccv = ccb[:, g * GCW:(g + 1) * GCW].rearrange(
                        "p (m c) -> p m c", c=25
                    )
                    nc.vector.tensor_add(tyv, tyv, ccv)

                nc.gpsimd.dma_start(out_o[b], outb[:])
    if os.environ.get("BRB_SKIP_SPLIT") != "1":
        _split_multi_waits(nc)
    return nc


_NC_CACHE = {}


def _get_program():
    key = str(_MM_DT)
    if key not in _NC_CACHE:
        _NC_CACHE[key] = _build_program()
    return _NC_CACHE[key]


def _host_fold(inputs):
    """Fold BN into conv weights/biases; build per-core input maps."""
    f = lambda k: np.asarray(inputs[k], dtype=np.float32)
    W0, b0, g0, be0, m0, v0 = (f(k) for k in ("W0", "b0", "g0", "be0", "m0", "v0"))
    W1, b1, g1, be1, m1, v1 = (f(k) for k in ("W1", "b1", "g1", "be1", "m1", "v1"))
    Wc, bc, Wr, br = (f(k) for k in ("Wc", "bc", "Wr", "br"))

    s0 = g0 / np.sqrt(v0 + BN_EPS)
    W0f = W0 * s0[:, None]
    b0f = (b0 - m0) * s0 + be0
    s1 = g1 / np.sqrt(v1 + BN_EPS)
    W1f = W1 * s1[:, None]
    b1f = (b1 - m1) * s1 + be1

    w0T = np.ascontiguousarray(W0f.T)            # [256, 128]
    wh = np.zeros((C1, CH), np.float32)          # [128, 32]
    wh[:, 0:NCLS] = Wc.T
    wh[:, NCLS:25] = Wr.T


    mm_np = mybir.dt.np(_MM_DT)
    fused = np.asarray(inputs["fused_feats"], dtype=np.float32)
    # [B, 256, N] -> [B, 2, 128, N] K-chunk split (a reshape, no copy)
    fused = fused.reshape(B, 2, 128, N).astype(mm_np)

    angle = np.asarray(inputs["coarse_angle"], dtype=np.float32)
    dist = np.asarray(inputs["coarse_distance"], dtype=np.float32)
    cc = np.empty((B, N, 25), np.float32)
    cc[:, :, 0:NCLS] = bc[None, None, :]
    cc[:, :, NCLS] = angle + br[0]
    cc[:, :, NCLS + 1:25] = dist + br[None, None, 1:]
    if _QUAD:
        # cc_dev[b, 32k+p, g*400 + m*25 + c] = cc[b, (4g+k)*512 + 32m + p, c]
        cc_dev = np.ascontiguousarray(
            cc.reshape(B, NTILES // 4, 4, NT // 32, 32, 25).transpose(
                0, 2, 4, 1, 3, 5
            )
        ).reshape(B, 128, (NTILES // 4) * (NT // 32) * 25)
    else:
        # cc_dev[b, p, t*400 + m*25 + c] = cc[b, t*512 + 32m + p, c]
        cc_dev = np.ascontiguousarray(
            cc.reshape(B, NTILES, NT // 32, 32, 25).transpose(0, 3, 1, 2, 4)
        ).reshape(B, 32, NTILES * (NT // 32) * 25)

    shared = {
        "w0a": np.ascontiguousarray(w0T[0:128]).astype(mm_np),
        "w0b": np.ascontiguousarray(w0T[128:256]).astype(mm_np),
        "w1": np.ascontiguousarray(W1f.T).astype(mm_np),
        "wh": wh.astype(mm_np),
        "b0": b0f.reshape(C1, 1),
        "b1": b1f.reshape(C1, 1),
    }
    in_maps = []
    for c in range(N_CORES):
        lo, hi = c * BPC, (c + 1) * BPC
        m = dict(shared)
        m["x"] = np.ascontiguousarray(fused[lo:hi])
        m["cc"] = np.ascontiguousarray(cc_dev[lo:hi])
        in_maps.append(m)
    return in_maps


def _run(inputs, trace=False):
    nc = _get_program()
    in_maps = _host_fold(inputs)
    res = run_bass_kernel_spmd(
        nc, in_maps, core_ids=list(range(N_CORES)), trace=trace
    )
    raw = np.concatenate([res.results[c]["out"] for c in range(N_CORES)], axis=0)
    if _QUAD:
        # raw[b, 32k+p, g*512+32m+c] = channel c at pos (4g+k)*512 + 32m + p
        full = np.ascontiguousarray(
            raw.reshape(B, 4, 32, NTILES // 4, NT // 32, 32).transpose(
                0, 3, 1, 4, 2, 5
            )
        ).reshape(B, N, 32)
    else:
        # raw[b, p, t*512 + 32m + c] = channel c at pos t*512 + 32m + p
        full = np.ascontiguousarray(
            raw.reshape(B, 32, NTILES, NT // 32, 32).transpose(0, 2, 3, 1, 4)
        ).reshape(B, N, 32)
    sem_scores = np.ascontiguousarray(full[:, :, 0:NCLS])
    refined_angle = np.ascontiguousarray(full[:, :, NCLS])
    refined_distance = np.ascontiguousarray(full[:, :, NCLS + 1:25])
    return (sem_scores, refined_angle, refined_distance), res


def kernel(**inputs):
    outs, _ = _run(inputs, trace=False)
    return outs
